# revision 1
# baseline (speedup 1.0000x reference)
"""3x3 morphological dilation (== 3x3 stride-1 max-pool) on Trainium2.

Input:  img [16, 8, 512, 512] f32 in [0, 1).
Output: out[b,c,y,x] = max over the 3x3 window of img (border padded with -2,
        which never wins since img >= 0 -- so replicate padding is equivalent).

Strategy (8 NeuronCores, pure data parallel over H):
  - Host slices each core an overlapping horizontal strip of ALL 128 (b,c)
    images: rows [64k-1 .. 64k+64] (66 rows, edge rows replicated at the
    global top/bottom which is max-equivalent to -2 padding).
  - On-core layout: partition dim = 128 (b*c) images, free dim = rows x cols.
  - Per R-output-row tile: load [128, R+2, 512] once (HWDGE on SP), vertical
    3-tap max via the pairwise trick (1.5 ops/elem) full-width, horizontal
    3-tap max via the pairwise trick per 256-col half, all fp32-exact
    tensor_tensor(max) on the Vector engine. Stores go out on the ACT HWDGE
    ring so store-waits never head-of-line-block the next load.
  - First/last tiles are small (8 rows) to shorten pipeline fill/drain.
  - vm has persistent border columns (x=-1 and x=512) memset once to -2.
"""

import numpy as np

import concourse.bass as bass
import concourse.tile as tile
from concourse import bacc, mybir
from concourse.bass_utils import run_bass_kernel_spmd

N_CORES = 8
B, C, H, W = 16, 8, 512, 512
NIMG = B * C                     # 128 -> partition dim
ROWS_PER_CORE = H // N_CORES     # 64
STRIP_ROWS = ROWS_PER_CORE + 2   # 66 (1 halo row each side)
TILE_PLAN = (8, 16, 16, 16, 8)   # output rows per tile (sums to 64)
HALF_W = 256
VM_W = 516                       # 514 cols used (x=-1..512 padded) + 2 align
F32 = mybir.dt.float32

_compiled = {}


def _build_nc():
    nc = bacc.Bacc(
        "TRN2",
        target_bir_lowering=False,
        debug=False,
        num_devices=N_CORES,
    )
    img = nc.dram_tensor(
        "img", [NIMG, STRIP_ROWS, W], F32, kind="ExternalInput"
    ).ap()
    out = nc.dram_tensor(
        "out", [NIMG, ROWS_PER_CORE, W], F32, kind="ExternalOutput"
    ).ap()

    max_r = max(TILE_PLAN)
    with tile.TileContext(nc) as tc:
        with (
            tc.tile_pool(name="pin", bufs=2) as pin,
            tc.tile_pool(name="pwork", bufs=1) as pwork,
            tc.tile_pool(name="pout", bufs=3) as pout,
        ):
            # Persistent scratch: vm rows/cols are rewritten every tile, but
            # the -2 border columns (vm col 0 = x=-1, col 513 = x=512) are
            # written once here and never touched again.
            p = pwork.tile([NIMG, max_r // 2 + 1, W], F32)
            vm = pwork.tile([NIMG, max_r, VM_W], F32)
            hp = pwork.tile([NIMG, max_r, 132], F32)
            nc.vector.memset(vm[:, :, 0:1], -2.0)
            nc.vector.memset(vm[:, :, 513:514], -2.0)

            r0 = 0
            for ti, R in enumerate(TILE_PLAN):
                npair = R // 2 + 1
                # Load strip rows r0 .. r0+R+1 (L[0..R+1]); tile's outputs
                # are strip rows r0+1 .. r0+R (= global out rows r0..r0+R-1).
                tin = pin.tile([NIMG, max_r + 2, W], F32, tag="tin")
                if ti == 0:
                    # Split the very first load (and its vertical pass) into
                    # two row chunks so DVE work starts as early as possible.
                    assert R == 8
                    nc.sync.dma_start(tin[:, 0:6, :], img[:, 0:6, :])
                    nc.sync.dma_start(tin[:, 6:10, :], img[:, 6:10, :])
                    # chunk A: vmax rows j=1..4 from L[0..5]
                    nc.vector.tensor_max(
                        p[:, 0:3, :], tin[:, 0:6:2, :], tin[:, 1:6:2, :]
                    )
                    nc.vector.tensor_max(
                        vm[:, 0:4:2, 1:513], p[:, 0:2, :], tin[:, 2:6:2, :]
                    )
                    nc.vector.tensor_max(
                        vm[:, 1:4:2, 1:513], tin[:, 1:4:2, :], p[:, 1:3, :]
                    )
                    # chunk B: vmax rows j=5..8 from L[4..9]
                    nc.vector.tensor_max(
                        p[:, 3:5, :], tin[:, 6:10:2, :], tin[:, 7:10:2, :]
                    )
                    nc.vector.tensor_max(
                        vm[:, 4:8:2, 1:513], p[:, 2:4, :], tin[:, 6:10:2, :]
                    )
                    nc.vector.tensor_max(
                        vm[:, 5:8:2, 1:513], tin[:, 5:8:2, :], p[:, 3:5, :]
                    )
                else:
                    nc.sync.dma_start(
                        tin[:, 0 : R + 2, :], img[:, r0 : r0 + R + 2, :]
                    )
                    # --- vertical 3-tap max, full width (pairwise trick) ---
                    # P[i] = max(L[2i], L[2i+1])           i = 0..R/2
                    # vmax[2i+1] = max(P[i], L[2i+2])      i = 0..R/2-1
                    # vmax[2i]   = max(L[2i-1], P[i])      i = 1..R/2
                    # vmax[j] -> vm row j-1; vm col x+1 <-> padded col x.
                    nc.vector.tensor_max(
                        p[:, 0:npair, :],
                        tin[:, 0 : R + 2 : 2, :],
                        tin[:, 1 : R + 2 : 2, :],
                    )
                    nc.vector.tensor_max(
                        vm[:, 0:R:2, 1:513],
                        p[:, 0 : npair - 1, :],
                        tin[:, 2 : R + 2 : 2, :],
                    )
                    nc.vector.tensor_max(
                        vm[:, 1:R:2, 1:513],
                        tin[:, 1 : R : 2, :],
                        p[:, 1:npair, :],
                    )

                # --- horizontal 3-tap max per 256-col half (pairwise) ---
                # window for out col lx (x = base+lx) = vm cols
                # {base+lx, base+lx+1, base+lx+2}
                # HP[j] = max(vm[base+2j], vm[base+2j+1])   j = 0..128
                # out[2j]   = max(HP[j], vm[base+2j+2])     j = 0..127
                # out[2j+1] = max(vm[base+2j+1], HP[j+1])   j = 0..127
                # For the last tile, additionally split the horizontal pass
                # and store by row-halves so the final store drains early.
                last = r0 + R == ROWS_PER_CORE
                row_chunks = (
                    [(0, R // 2), (R // 2, R)] if last and R > 2 else [(0, R)]
                )
                for h in range(2):
                    base = h * HALF_W
                    for ra, rb in row_chunks:
                        o = pout.tile([NIMG, max_r, HALF_W], F32, tag="o")
                        nc.vector.tensor_max(
                            hp[:, ra:rb, 0:129],
                            vm[:, ra:rb, base : base + 258 : 2],
                            vm[:, ra:rb, base + 1 : base + 258 : 2],
                        )
                        nc.vector.tensor_max(
                            o[:, ra:rb, 0:256:2],
                            hp[:, ra:rb, 0:128],
                            vm[:, ra:rb, base + 2 : base + 258 : 2],
                        )
                        nc.vector.tensor_max(
                            o[:, ra:rb, 1:256:2],
                            vm[:, ra:rb, base + 1 : base + 257 : 2],
                            hp[:, ra:rb, 1:129],
                        )
                        nc.scalar.dma_start(
                            out[:, r0 + ra : r0 + rb, base : base + HALF_W],
                            o[:, ra:rb, :],
                        )
                r0 += R

    nc.compile()
    return nc


def _get_nc():
    if "nc" not in _compiled:
        _compiled["nc"] = _build_nc()
    return _compiled["nc"]


def _make_shards(flat: np.ndarray) -> list[np.ndarray]:
    """flat: [128, 512, 512] -> 8 strips of [128, 66, 512] with 1-row halo,
    edge-replicated at the global top/bottom."""
    shards = []
    for k in range(N_CORES):
        lo = k * ROWS_PER_CORE - 1
        hi = k * ROWS_PER_CORE + ROWS_PER_CORE + 1
        if lo < 0:
            strip = np.concatenate([flat[:, :1], flat[:, 0:hi]], axis=1)
        elif hi > H:
            strip = np.concatenate([flat[:, lo:], flat[:, H - 1 :]], axis=1)
        else:
            strip = flat[:, lo:hi]
        shards.append(np.ascontiguousarray(strip, dtype=np.float32))
    return shards


def kernel(img: np.ndarray, **_unused) -> np.ndarray:
    img = np.asarray(img, dtype=np.float32)
    assert img.shape == (B, C, H, W), img.shape
    flat = img.reshape(NIMG, H, W)

    nc = _get_nc()
    in_maps = [{"img": s} for s in _make_shards(flat)]
    res = run_bass_kernel_spmd(nc, in_maps, core_ids=list(range(N_CORES)))
    parts = [res.results[k]["out"] for k in range(N_CORES)]
    full = np.concatenate(parts, axis=1)  # [128, 512, 512]
    return full.reshape(B, C, H, W).astype(np.float32, copy=False)



# revision 4
# speedup vs baseline: 1.6949x; 1.6949x over previous
"""3x3 morphological dilation (== 3x3 stride-1 max-pool) on Trainium2.

Input:  img [16, 8, 512, 512] f32 in [0, 1).
Output: out[b,c,y,x] = max over the 3x3 window of img (border padded with -2,
        which never wins since img >= 0 -- so replicate padding is equivalent).

Strategy (8 NeuronCores, pure data parallel over H), v3 -- fp16 + column
de-interleave so every DVE max op runs in the 2x_1P perf mode:
  - All device I/O and compute in fp16. max() is exact on fp16 values, so the
    only error vs the f32 reference is input rounding (rel <= 2^-11 ~ 5e-4),
    far inside the 2e-2 gate. Halves HBM traffic (the memory roofline) and
    doubles DVE throughput -- but 2x_1P needs 16-bit dtype, innermost step
    +-1 AND 4B alignment, which plain horizontal 3-tap shifts break.
  - Fix: the HOST de-interleaves columns per row: strip[.., 0:256] = even
    cols x=0,2,..,510 (E plane), strip[.., 256:512] = odd cols (O plane).
    Vertical 3-tap max runs full-width (pairwise trick, 1.5 ops/elem, 2x)
    producing vmP = [E-plane vm | O-plane vm]. Horizontal then becomes
      T[j]    = max(vmE[j], vmO[j])        # dense, aligned, 2x
      outE[j] = max(T[j], vmO[j-1])        # needs 1-elem shifted vmO
      outO[j] = max(T[j], vmE[j+1])        # needs 1-elem shifted vmE
    The two shifted (misaligned) copies OS/ES are made by the otherwise-idle
    Scalar/ACT engine, so ALL DVE tensor_tensor ops stay aligned at 2x.
    Boundary cols use OS[0] = ES[255] = -2 (memset once; never wins).
  - outE/outO are stored as planes (full-width contiguous store on the ACT
    HWDGE ring); the host re-interleaves columns of the result.
  - Software pipelining: tile t's outE/outO are emitted after tile t+1's
    vertical+T ops, giving ACT a full tile of slack to produce OS/ES.
  - Host slices each core an overlapping strip of ALL 128 (b,c) images:
    rows [64k-1 .. 64k+64] (66 rows, edge rows replicated at the global
    top/bottom which is max-equivalent to -2 padding).
"""

import numpy as np

import concourse.bass as bass
import concourse.tile as tile
from concourse import bacc, mybir
from concourse.bass_utils import run_bass_kernel_spmd

N_CORES = 8
B, C, H, W = 16, 8, 512, 512
NIMG = B * C                     # 128 -> partition dim
HW_ = W // 2                     # 256 cols per plane
ROWS_PER_CORE = H // N_CORES     # 64
STRIP_ROWS = ROWS_PER_CORE + 2   # 66 (1 halo row each side)
TILE_PLAN = (8, 16, 16, 16, 8)   # output rows per tile (sums to 64)
F16 = mybir.dt.float16

_compiled = {}


def _build_nc():
    nc = bacc.Bacc(
        "TRN2",
        target_bir_lowering=False,
        debug=False,
        num_devices=N_CORES,
    )
    img = nc.dram_tensor(
        "img", [NIMG, STRIP_ROWS, W], F16, kind="ExternalInput"
    ).ap()
    out = nc.dram_tensor(
        "out", [NIMG, ROWS_PER_CORE, W], F16, kind="ExternalOutput"
    ).ap()

    max_r = max(TILE_PLAN)
    with tile.TileContext(nc) as tc:
        with (
            tc.tile_pool(name="pin", bufs=2) as pin,
            tc.tile_pool(name="pwork", bufs=1) as pwork,
            tc.tile_pool(name="pout", bufs=3) as pout,
        ):
            # Persistent double-buffered scratch (cross-engine users), plus
            # single-buffer p (DVE-only, serial use).
            p = pwork.tile([NIMG, max_r // 2 + 1, W], F16)
            vmps = [
                pwork.tile([NIMG, max_r, W], F16, name=f"vmp{i}")
                for i in range(2)
            ]
            tbufs = [
                pwork.tile([NIMG, max_r, HW_], F16, name=f"tb{i}")
                for i in range(2)
            ]
            oss = [
                pwork.tile([NIMG, max_r, HW_], F16, name=f"os{i}")
                for i in range(2)
            ]
            ess = [
                pwork.tile([NIMG, max_r, HW_], F16, name=f"es{i}")
                for i in range(2)
            ]
            for i in range(2):
                # Border cols: vm[x=-1] and vm[x=512] surrogates; -2 never
                # wins. Written once; the per-tile ACT copies never touch
                # these columns.
                nc.vector.memset(oss[i][:, :, 0:1], -2.0)
                nc.vector.memset(ess[i][:, :, HW_ - 1 : HW_], -2.0)

            def phase1(ti, R, r0):
                """load + vertical pairwise + T + ACT shifted copies."""
                npair = R // 2 + 1
                vmp, tb = vmps[ti % 2], tbufs[ti % 2]
                os_, es_ = oss[ti % 2], ess[ti % 2]
                tin = pin.tile([NIMG, max_r + 2, W], F16, tag="tin")
                nc.sync.dma_start(
                    tin[:, 0 : R + 2, :], img[:, r0 : r0 + R + 2, :]
                )
                # vertical 3-tap max over rows, full width (both planes),
                # pairwise: P[i] = max(L[2i], L[2i+1]);
                # vm[2i+1] = max(P[i], L[2i+2]); vm[2i] = max(L[2i-1], P[i]).
                nc.vector.tensor_max(
                    p[:, 0:npair, :],
                    tin[:, 0 : R + 2 : 2, :],
                    tin[:, 1 : R + 2 : 2, :],
                )
                nc.vector.tensor_max(
                    vmp[:, 0:R:2, :],
                    p[:, 0 : npair - 1, :],
                    tin[:, 2 : R + 2 : 2, :],
                )
                nc.vector.tensor_max(
                    vmp[:, 1:R:2, :],
                    tin[:, 1:R:2, :],
                    p[:, 1:npair, :],
                )
                # T = within-pair max (dense, 2x)
                nc.vector.tensor_max(
                    tb[:, 0:R, :], vmp[:, 0:R, 0:HW_], vmp[:, 0:R, HW_:W]
                )
                # ACT: shifted planes. OS[j] = vmO[j-1], ES[j] = vmE[j+1].
                nc.scalar.copy(
                    os_[:, 0:R, 1:HW_], vmp[:, 0:R, HW_ : W - 1]
                )
                nc.scalar.copy(
                    es_[:, 0:R, 0 : HW_ - 1], vmp[:, 0:R, 1:HW_]
                )

            def phase2(ti, R, r0):
                """outE/outO + store."""
                tb = tbufs[ti % 2]
                os_, es_ = oss[ti % 2], ess[ti % 2]
                o = pout.tile([NIMG, max_r, W], F16, tag="o")
                nc.vector.tensor_max(
                    o[:, 0:R, 0:HW_], tb[:, 0:R, :], os_[:, 0:R, :]
                )
                nc.vector.tensor_max(
                    o[:, 0:R, HW_:W], tb[:, 0:R, :], es_[:, 0:R, :]
                )
                nc.scalar.dma_start(
                    out[:, r0 : r0 + R, :], o[:, 0:R, :]
                )

            starts = []
            r0 = 0
            for R in TILE_PLAN:
                starts.append(r0)
                r0 += R
            prev = None
            for ti, R in enumerate(TILE_PLAN):
                phase1(ti, R, starts[ti])
                if prev is not None:
                    phase2(*prev)
                prev = (ti, R, starts[ti])
            phase2(*prev)

    nc.compile()
    return nc


def _get_nc():
    if "nc" not in _compiled:
        _compiled["nc"] = _build_nc()
    return _compiled["nc"]


def _prep(img: np.ndarray) -> list[dict[str, np.ndarray]]:
    """f32 [16,8,512,512] -> 8 de-interleaved fp16 halo strips."""
    flat = np.asarray(img, dtype=np.float32).reshape(NIMG, H, W)
    flat = flat.astype(np.float16)
    di = np.empty_like(flat)
    di[:, :, 0:HW_] = flat[:, :, 0::2]
    di[:, :, HW_:W] = flat[:, :, 1::2]
    shards = []
    for k in range(N_CORES):
        lo = k * ROWS_PER_CORE - 1
        hi = k * ROWS_PER_CORE + ROWS_PER_CORE + 1
        if lo < 0:
            strip = np.concatenate([di[:, :1], di[:, 0:hi]], axis=1)
        elif hi > H:
            strip = np.concatenate([di[:, lo:], di[:, H - 1 :]], axis=1)
        else:
            strip = di[:, lo:hi]
        shards.append(np.ascontiguousarray(strip, dtype=np.float16))
    return [{"img": s} for s in shards]


def _post(parts: list[np.ndarray]) -> np.ndarray:
    """8x fp16 [128,64,512] plane outputs -> f32 [16,8,512,512]."""
    di = np.concatenate(parts, axis=1)  # [128, 512, 512] as [E | O] planes
    full = np.empty((NIMG, H, W), dtype=np.float16)
    full[:, :, 0::2] = di[:, :, 0:HW_]
    full[:, :, 1::2] = di[:, :, HW_:W]
    return full.reshape(B, C, H, W).astype(np.float32)


def kernel(img: np.ndarray, **_unused) -> np.ndarray:
    img = np.asarray(img, dtype=np.float32)
    assert img.shape == (B, C, H, W), img.shape
    nc = _get_nc()
    res = run_bass_kernel_spmd(nc, _prep(img), core_ids=list(range(N_CORES)))
    return _post([res.results[k]["out"] for k in range(N_CORES)])
